# revision 13
# baseline (speedup 1.0000x reference)
"""Cross-attention (b=1, n=2048, dim=1024, 16 heads x 64) on 8 TRN2 NeuronCores.

Strategy (v3):
- Tensor-parallel over heads: core k computes heads (2k, 2k+1) end to end and a
  partial output projection; host sums the 8 partials (the Wo all-reduce).
- Mask compaction on host: masked ROWS get uniform attention = (mean v) @ Wo
  computed on host; masked COLUMNS excluded from the C0 unmasked positions
  (padded to a multiple of 128), roughly halving all n^2 work.
- fp16 matmul datapath, fp32 PSUM accumulation, fp16 partials summed in
  float64 on host (all fp8 placements measured 1.1-3.7% rel err in host
  emulation -- too close to the 2e-2 gate).
- Softmax denominator FUSED into the P@V stationary as a 65th all-ones
  column (removes the separate ones-matmul, -16k PE cycles).  Normalize:
  den row -> SBUF copy -> DVE reciprocal -> gpsimd partition_broadcast ->
  one DVE multiply per head.  (The custom recip op misreads PSUM at a
  partition offset, hence the copy; DVE ops may read only one PSUM operand,
  hence the broadcast to SBUF.)
- THREE i-chunks (512, 384, 128): the last chunk is small so the
  exp->pv->finish->Wo->store chain after the final exp is short.
- PE kept dense to hold the HAM clock at 8/8: 512-wide projection streams,
  S-issues spread between independent work, dummy matmuls filling known
  DMA-wait gaps (an idle PE drops to 4/8 and halves throughput for ~8us).
- Input DMA striped over both HWDGE queues in ~0.5MB pieces ordered by
  earliest need; gpsimd SWDGE carries jbias/wv/wo only, after its memsets.
- All output stores ride the sync queue per 512-col block (scalar engine is
  kept free of DMA triggers so exps and tail evictions are never delayed).
"""
import numpy as np

N_CORES = 8
HEADS = 16
DH = 64  # head dim
DIM = 1024
HPC = HEADS // N_CORES  # heads per core = 2
CB = DIM // 128  # contraction blocks for projections (8)

_cache = {}


def _build(C, JB, chunks):
    """Build + schedule the per-core Bass program for padded length C=1024."""
    import concourse.mybir as mybir
    import concourse.tile as tile
    from concourse import bacc
    from concourse.masks import make_identity

    F32 = mybir.dt.float32
    F16 = mybir.dt.float16
    EXP = mybir.ActivationFunctionType.Exp
    scale = DIM ** -0.5
    HW = C // 2  # half width (512)

    nc = bacc.Bacc("TRN2", target_bir_lowering=False, debug=False)

    x_d = nc.dram_tensor("x16", [128, CB * C], F16, kind="ExternalInput").ap()
    m_d = nc.dram_tensor("m16", [128, CB * C], F16, kind="ExternalInput").ap()
    wq_d = nc.dram_tensor("wq", [128, DIM], F16, kind="ExternalInput").ap()
    wk_d = nc.dram_tensor("wk", [128, DIM], F16, kind="ExternalInput").ap()
    wv_d = nc.dram_tensor("wv", [128, DIM], F16, kind="ExternalInput").ap()
    wo_d = nc.dram_tensor("wo", [128, DIM], F16, kind="ExternalInput").ap()
    jb_d = nc.dram_tensor("jbias", [128, JB], F32, kind="ExternalInput").ap()
    out_d = nc.dram_tensor("out", [C, DIM], F16, kind="ExternalOutput").ap()

    NCH = len(chunks)

    with tile.TileContext(nc) as tc:
        with (
            tc.tile_pool(name="persist", bufs=1) as pp,
            tc.tile_pool(name="outstage", bufs=6) as outp,
        ):
            # ---- persistent tiles ----
            xT = pp.tile([128, 2, CB, HW], F16)  # half-major
            mT = pp.tile([128, 2, CB, HW], F16)
            qT = pp.tile([128, C], F16)  # [d(2 heads), i]
            kT = pp.tile([128, C], F16)
            vTs = pp.tile([128, C], F16)
            # v natural layout + fused denominator column:
            # [j-in-block, jb, head, 64 v-cols + 1 ones-col]
            v1e = pp.tile([128, JB, HPC, DH + 1], F16)
            onesw = pp.tile([128, DH], F16)
            dummy = pp.tile([128, 512], F16)
            ident = pp.tile([128, 128], F16)
            wo_sb = pp.tile([128, DIM], F16)
            wq_sb = pp.tile([128, CB, 128], F16)
            wk_sb = pp.tile([128, CB, 128], F16)
            wv_sb = pp.tile([128, CB, 128], F16)
            jbias = pp.tile([128, JB], F32)
            ON = pp.tile([128, C], F16)  # normalized attn out^T (both heads)
            PT = pp.tile([128, NCH, JB, HPC, 512], F16)

            # ---- gpsimd: memsets FIRST so PE warmup is not stuck behind
            # the slow SWDGE triggers; SWDGE DMAs after.
            nc.gpsimd.memset(onesw[:], 1.0)
            make_identity(nc, ident[:])
            nc.vector.memset(dummy[:], 0.001)
            nc.vector.memset(v1e[:, :, :, DH : DH + 1], 1.0)

            # ---- loads ----
            # x/m half-major [128, half, cb, 512]; pieces of 0.5MB (4 cb),
            # two HWDGE queues, ordered by earliest need.
            xr = x_d.rearrange("p (hf cb i) -> p hf cb i", hf=2, cb=CB)
            mr = m_d.rearrange("p (hf cb i) -> p hf cb i", hf=2, cb=CB)
            nc.sync.dma_start(wk_sb[:], wk_d.rearrange("p (cb d) -> p cb d", cb=CB))
            nc.scalar.dma_start(wq_sb[:], wq_d.rearrange("p (cb d) -> p cb d", cb=CB))
            nc.sync.dma_start(xT[:, 0, 0:4], xr[:, 0, 0:4])
            nc.scalar.dma_start(xT[:, 0, 4:8], xr[:, 0, 4:8])
            nc.sync.dma_start(mT[:, 0, 0:4], mr[:, 0, 0:4])
            nc.scalar.dma_start(mT[:, 0, 4:8], mr[:, 0, 4:8])
            nc.sync.dma_start(mT[:, 1, 0:4], mr[:, 1, 0:4])
            nc.scalar.dma_start(mT[:, 1, 4:8], mr[:, 1, 4:8])
            nc.sync.dma_start(xT[:, 1, 0:4], xr[:, 1, 0:4])
            nc.scalar.dma_start(xT[:, 1, 4:8], xr[:, 1, 4:8])
            # SWDGE (slow ~50GB/s): small/late-needed tensors only
            nc.gpsimd.dma_start(jbias[:], jb_d)
            nc.gpsimd.dma_start(wv_sb[:], wv_d.rearrange("p (cb d) -> p cb d", cb=CB))
            nc.gpsimd.dma_start(wo_sb[:], wo_d)

            # ---------- helpers ----------
            def s_pair(ci, i0, cw, jb, sps):
                for h in range(HPC):
                    nc.tensor.matmul(
                        sps[:, h, :cw],
                        kT[h * DH : (h + 1) * DH, jb * 128 : (jb + 1) * 128],
                        qT[h * DH : (h + 1) * DH, i0 : i0 + cw],
                        start=True,
                        stop=True,
                    )
                with nc.allow_low_precision(reason="softmax weights fp16"):
                    nc.scalar.activation(
                        PT[:, ci, jb, :, :cw],
                        sps[:, :, :cw],
                        EXP,
                        bias=jbias[:, jb : jb + 1],
                        scale=scale,
                    )

            def pv_pair(ci, jb, opsT):
                cw = chunks[ci][1]
                # 65-row output per head: rows 0-63 = P@V, row 64 = denom
                for h in range(HPC):
                    nc.tensor.matmul(
                        opsT[0 : DH + 1, h, :cw],
                        v1e[:, jb, h],
                        PT[:, ci, jb, h, :cw],
                        start=(jb == 0),
                        stop=(jb == JB - 1),
                    )

            def finish_chunk(ci, opsT, den32, rec32, recd_sb):
                i0, cw = chunks[ci]
                # copy both heads' denominator rows PSUM->SBUF (the custom
                # recip op misreads PSUM at a partition offset), then recip
                nc.vector.tensor_copy(den32[0:1, :, :cw], opsT[DH : DH + 1, :, :cw])
                nc.vector.reciprocal_approx_fast(
                    rec32[0:1, :, :cw], den32[0:1, :, :cw]
                )
                nc.gpsimd.partition_broadcast(
                    recd_sb[:, :, :cw], rec32[0:1, :, :cw]
                )
                with nc.allow_low_precision(reason="attn out fp16"):
                    for h in range(HPC):
                        nc.vector.tensor_mul(
                            ON[h * DH : (h + 1) * DH, i0 : i0 + cw],
                            opsT[0:DH, h, :cw],
                            recd_sb[h * DH : (h + 1) * DH, h, :cw],
                        )

            def wo_isub(isub, psE, evicts, st_engs):
                ob = outp.tile([128, DIM], F16, tag="ob")
                for eb in range(DIM // 512):
                    dp = psE.tile([128, 512], F32, tag="dout")
                    nc.tensor.matmul(
                        dp[:],
                        ON[:, isub * 128 : (isub + 1) * 128],
                        wo_sb[:, eb * 512 : (eb + 1) * 512],
                        start=True,
                        stop=True,
                    )
                    with nc.allow_low_precision(reason="partial out fp16"):
                        evicts[eb % len(evicts)](
                            ob[:, eb * 512 : (eb + 1) * 512], dp[:]
                        )
                    # during attention all stores ride the sync queue (the
                    # scalar engine must never stall exps on DMA triggers);
                    # in the tail (after the last exp) they pair up
                    st_engs[eb % len(st_engs)].dma_start(
                        out_d[isub * 128 : (isub + 1) * 128, eb * 512 : (eb + 1) * 512],
                        ob[:, eb * 512 : (eb + 1) * 512],
                    )

            with (
                tc.tile_pool(name="psS", bufs=2, space="PSUM") as psS,
                tc.tile_pool(name="nrm", bufs=2) as nrm,
            ):
                slist = [(ci, i0, cw, jb) for ci, (i0, cw) in enumerate(chunks)
                         for jb in range(JB)]
                si = 0

                def issue_s():
                    nonlocal si
                    ci, i0, cw, jb = slist[si]
                    sps = psS.tile([128, HPC, 512], F32, tag="S")
                    s_pair(ci, i0, cw, jb, sps)
                    si += 1

                with (
                    tc.tile_pool(name="psP", bufs=2, space="PSUM") as psP,
                    tc.tile_pool(name="psQ", bufs=1, space="PSUM") as psQ,
                    tc.tile_pool(name="psT", bufs=1, space="PSUM") as psT,
                ):
                    # warm up the PE clock while loads stream
                    dmt = psQ.tile([128, 512], F32, tag="projq", name="dummy_ps")
                    for t in range(12):
                        nc.tensor.matmul(
                            dmt[0:DH, :], onesw[:], dummy[:],
                            start=(t == 0), stop=(t == 11),
                        )

                    def fill(n):
                        # dummy matmuls to keep the PE busy (and the HAM
                        # clock at 8/8) across known DMA-wait gaps
                        for t in range(n):
                            nc.tensor.matmul(
                                dmt[0:DH, :], onesw[:], dummy[:],
                                start=(t == 0), stop=(t == n - 1),
                            )

                    pps = {}

                    def _proj_half(hf, w_sb, dst, nm, cb0, cb1):
                        key = (nm, hf)
                        if key not in pps:
                            pps[key] = psP.tile([128, HW], F32, tag="projkv",
                                                name=f"p{nm}{hf}")
                        pq_ = pps[key]
                        for cb in range(cb0, cb1):
                            nc.tensor.matmul(
                                pq_[:],
                                w_sb[:, cb, :],
                                mT[:, hf, cb, :],
                                start=(cb == 0),
                                stop=(cb == CB - 1),
                            )
                        if cb1 == CB:
                            nc.vector.tensor_copy(
                                dst[:, hf * HW : (hf + 1) * HW], pq_[:]
                            )
                            del pps[key]

                    def k_half(hf, cb0=0, cb1=CB):
                        _proj_half(hf, wk_sb, kT, "k", cb0, cb1)

                    def v_half(hf, cb0=0, cb1=CB):
                        _proj_half(hf, wv_sb, vTs, "v", cb0, cb1)

                    ptt = psT.tile([128, 2, 128], F16, tag="vt")

                    def t_quarter(q):
                        for k, jb in enumerate((2 * q, 2 * q + 1)):
                            nc.tensor.transpose(
                                ptt[:, k, :], vTs[:, jb * 128 : (jb + 1) * 128],
                                ident[:],
                            )
                        nc.vector.tensor_copy(
                            v1e[:, 2 * q : 2 * q + 2, :, 0:DH],
                            ptt[:].rearrange("p a (h d) -> p a h d", h=HPC),
                        )

                    qps = {}

                    def q_chunk(ci, cb0, cb1):
                        i0, cw = chunks[ci]
                        hf = i0 // HW
                        o0 = i0 - hf * HW
                        if ci not in qps:
                            qps[ci] = psQ.tile([128, 512], F32, tag="projq",
                                               name=f"pq{ci}")
                        pq_ = qps[ci]
                        for cb in range(cb0, cb1):
                            nc.tensor.matmul(
                                pq_[:, :cw],
                                wq_sb[:, cb, :],
                                xT[:, hf, cb, o0 : o0 + cw],
                                start=(cb == 0),
                                stop=(cb == CB - 1),
                            )
                        if cb1 == CB:
                            nc.vector.tensor_copy(qT[:, i0 : i0 + cw], pq_[:, :cw])

                    # ---- projection phase; S-issues spread so the in-order
                    # PE never camps long on a blocked instruction, dummy
                    # fills sized to the known DMA arrival gaps
                    fill(4)
                    q_chunk(0, 0, 4)   # x0a (sync)
                    q_chunk(0, 4, 8)   # x0b (scalar)
                    fill(8)
                    k_half(0, 0, 4)    # m0a
                    k_half(0, 4, 8)    # m0b
                    issue_s()   # S[0] c0 jb0
                    issue_s()   # S[1]
                    v_half(0)          # wv (SWDGE) + m0
                    issue_s()   # S[2]
                    t_quarter(0)
                    issue_s()   # S[3]
                    t_quarter(1)
                    k_half(1, 0, 4)    # m1a
                    k_half(1, 4, 8)    # m1b
                    issue_s()   # S[4]
                    issue_s()   # S[5]
                    v_half(1)
                    issue_s()   # S[6]
                    t_quarter(2)
                    issue_s()   # S[7]
                    t_quarter(3)
                    q_chunk(1, 0, 8)   # x half 1
                    q_chunk(2, 0, 8)

                with (
                    tc.tile_pool(name="psO", bufs=1, space="PSUM") as psO,
                    tc.tile_pool(name="psE", bufs=2, space="PSUM") as psE,
                ):
                    opsT = psO.tile([128, HPC, 512], F32, tag="O")
                    den32 = nrm.tile([1, HPC, 512], F32, tag="den")
                    rec32 = nrm.tile([1, HPC, 512], F32, tag="rec")
                    recd_sb = nrm.tile([128, HPC, 512], F32, tag="recd")

                    vcp = nc.vector.tensor_copy
                    scp = nc.scalar.copy
                    plan = [
                        ("s",),                       # S[8] = c1 jb0
                        ("pv", 0, 0), ("pv", 0, 1),
                        ("s",),                       # S[9]
                        ("pv", 0, 2), ("pv", 0, 3),
                        ("s",),                       # S[10]
                        ("pv", 0, 4), ("pv", 0, 5),
                        ("s",),                       # S[11]
                        ("pv", 0, 6), ("pv", 0, 7),
                        ("s",),                       # S[12]
                        ("fin", 0), ("wo", 0, None), ("pv", 1, 0),
                        ("s",),                       # S[13]
                        ("wo", 1, None), ("pv", 1, 1),
                        ("s",),                       # S[14]
                        ("wo", 2, None), ("pv", 1, 2),
                        ("s",),                       # S[15]
                        ("wo", 3, None), ("pv", 1, 3),
                        ("s",),                       # S[16] = c2 jb0
                        ("pv", 1, 4),
                        ("s",),                       # S[17]
                        ("pv", 1, 5),
                        ("s",),                       # S[18]
                        ("pv", 1, 6),
                        ("s",),                       # S[19]
                        ("pv", 1, 7),
                        ("s",),                       # S[20]
                        ("fin", 1), ("wo", 4, None), ("pv", 2, 0),
                        ("s",),                       # S[21]
                        ("pv", 2, 1), ("pv", 2, 2),
                        ("s",),                       # S[22]
                        ("pv", 2, 3), ("pv", 2, 4),
                        ("s",),                       # S[23]
                        ("wo", 5, "both"), ("pv", 2, 5), ("pv", 2, 6),
                        ("wo", 6, "both"), ("pv", 2, 7),
                        ("fin", 2),
                        ("wo", 7, "both"),
                    ]
                    for step in plan:
                        if step[0] == "s":
                            issue_s()
                        elif step[0] == "pv":
                            pv_pair(step[1], step[2], opsT)
                        elif step[0] == "fin":
                            finish_chunk(step[1], opsT, den32, rec32, recd_sb)
                        elif step[0] == "wo":
                            if step[2] == "both":
                                wo_isub(step[1], psE, [vcp, scp],
                                        [nc.sync, nc.scalar])
                            else:
                                wo_isub(step[1], psE, [vcp], [nc.sync])

    nc.compile()
    return nc


def _get_program(C, JB, chunks):
    key = (C, JB, tuple(chunks))
    if key not in _cache:
        _cache[key] = _build(C, JB, chunks)
    return _cache[key]


def _swizzle_w(a):  # [DIM, X] -> [128, CB*X] partition-major
    X = a.shape[1]
    return np.ascontiguousarray(
        a.reshape(CB, 128, X).transpose(1, 0, 2).reshape(128, CB * X)
    ).astype(np.float16)


def _swizzle_h(a_t, C):  # [DIM, C] -> [128, 2*CB*(C//2)] half-major
    HW = C // 2
    return np.ascontiguousarray(
        a_t.reshape(CB, 128, 2, HW).transpose(1, 2, 0, 3).reshape(128, CB * C)
    ).astype(np.float16)


def kernel(x, m, mask, Wq, Wk, Wv, Wo, bo, _trace=False, _bass_results=None):
    from concourse.bass_utils import run_bass_kernel_spmd

    x = np.asarray(x)
    m = np.asarray(m)
    mask = np.asarray(mask)
    Wq, Wk, Wv, Wo, bo = (np.asarray(a, np.float32) for a in (Wq, Wk, Wv, Wo, bo))
    b, n, dim = x.shape
    assert (b, dim) == (1, DIM)

    pm = np.concatenate([np.array([True]), mask[0]])  # [n]
    sel = np.nonzero(pm)[0]
    C0 = len(sel)
    C = max(((C0 + 127) // 128) * 128, 1024)
    JB = C // 128
    assert C == 1024, f"schedule is specialized for C=1024, got C0={C0}"
    chunks = [(0, 512), (512, 384), (896, 128)]

    x_c = np.zeros((C, DIM), np.float32)
    x_c[:C0] = x[0][sel]
    m_c = np.zeros((C, DIM), np.float32)
    m_c[:C0] = m[0][sel]
    x_t = np.ascontiguousarray(x_c.T)  # [DIM, C]
    m_t = np.ascontiguousarray(m_c.T)

    x_sw = _swizzle_h(x_t, C)
    m_sw = _swizzle_h(m_t, C)

    jbias = np.zeros(C, np.float32)
    jbias[C0:] = -1e30
    jbias_t = np.ascontiguousarray(jbias.reshape(JB, 128).T)  # [128, JB]

    nc = _get_program(C, JB, chunks)

    in_maps = []
    for c in range(N_CORES):
        h0 = c * HPC * DH  # 128*c
        in_maps.append(
            {
                "x16": x_sw,
                "m16": m_sw,
                "wq": _swizzle_w(np.ascontiguousarray(Wq[:, h0 : h0 + 128])),
                "wk": _swizzle_w(np.ascontiguousarray(Wk[:, h0 : h0 + 128])),
                "wv": _swizzle_w(np.ascontiguousarray(Wv[:, h0 : h0 + 128])),
                "wo": np.ascontiguousarray(Wo[h0 : h0 + 128, :]).astype(np.float16),
                "jbias": jbias_t,
            }
        )

    res = run_bass_kernel_spmd(
        nc, in_maps, core_ids=list(range(N_CORES)), trace=_trace
    )
    if _bass_results is not None:
        _bass_results.append(res)

    acc = np.sum(
        np.stack([np.asarray(r["out"])[:C0].astype(np.float64) for r in res.results]),
        axis=0,
    )

    # host-side: masked rows get uniform attention over ALL positions
    mv = m[0].astype(np.float64).mean(axis=0)  # mean over all j of m
    mv_out = (mv @ Wv.astype(np.float64)) @ Wo.astype(np.float64)  # [dim]

    out = np.empty((n, DIM), np.float64)
    out[sel] = acc
    out[~pm] = mv_out
    out += bo.astype(np.float64)
    return out[None].astype(np.float32)


# revision 14
# speedup vs baseline: 1.0313x; 1.0313x over previous
"""Cross-attention (b=1, n=2048, dim=1024, 16 heads x 64) on 8 TRN2 NeuronCores.

Strategy (v3):
- Tensor-parallel over heads: core k computes heads (2k, 2k+1) end to end and a
  partial output projection; host sums the 8 partials (the Wo all-reduce).
- Mask compaction on host: masked ROWS get uniform attention = (mean v) @ Wo
  computed on host; masked COLUMNS excluded from the C0 unmasked positions
  (padded to a multiple of 128), roughly halving all n^2 work.
- fp16 matmul datapath, fp32 PSUM accumulation, fp16 partials summed in
  float64 on host (all fp8 placements measured 1.1-3.7% rel err in host
  emulation -- too close to the 2e-2 gate).
- Softmax denominator FUSED into the P@V stationary as a 65th all-ones
  column (removes the separate ones-matmul, -16k PE cycles).  Normalize:
  den row -> SBUF copy -> DVE reciprocal -> gpsimd partition_broadcast ->
  one DVE multiply per head.  (The custom recip op misreads PSUM at a
  partition offset, hence the copy; DVE ops may read only one PSUM operand,
  hence the broadcast to SBUF.)
- THREE i-chunks (512, 384, 128): the last chunk is small so the
  exp->pv->finish->Wo->store chain after the final exp is short.
- PE kept dense to hold the HAM clock at 8/8: 512-wide projection streams,
  S-issues spread between independent work, dummy matmuls filling known
  DMA-wait gaps (an idle PE drops to 4/8 and halves throughput for ~8us).
- Input DMA striped over both HWDGE queues in ~0.5MB pieces ordered by
  earliest need; gpsimd SWDGE carries jbias/wv/wo only, after its memsets.
- All output stores ride the sync queue per 512-col block (scalar engine is
  kept free of DMA triggers so exps and tail evictions are never delayed).
"""
import numpy as np

N_CORES = 8
HEADS = 16
DH = 64  # head dim
DIM = 1024
HPC = HEADS // N_CORES  # heads per core = 2
CB = DIM // 128  # contraction blocks for projections (8)

_cache = {}


def _build(C, JB, chunks):
    """Build + schedule the per-core Bass program for padded length C=1024."""
    import concourse.mybir as mybir
    import concourse.tile as tile
    from concourse import bacc
    from concourse.masks import make_identity

    F32 = mybir.dt.float32
    F16 = mybir.dt.float16
    EXP = mybir.ActivationFunctionType.Exp
    scale = DIM ** -0.5
    HW = C // 2  # half width (512)

    nc = bacc.Bacc("TRN2", target_bir_lowering=False, debug=False)

    x_d = nc.dram_tensor("x16", [128, CB * C], F16, kind="ExternalInput").ap()
    m_d = nc.dram_tensor("m16", [128, CB * C], F16, kind="ExternalInput").ap()
    wq_d = nc.dram_tensor("wq", [128, DIM], F16, kind="ExternalInput").ap()
    wk_d = nc.dram_tensor("wk", [128, DIM], F16, kind="ExternalInput").ap()
    wv_d = nc.dram_tensor("wv", [128, DIM], F16, kind="ExternalInput").ap()
    wo_d = nc.dram_tensor("wo", [128, DIM], F16, kind="ExternalInput").ap()
    jb_d = nc.dram_tensor("jbias", [128, JB], F32, kind="ExternalInput").ap()
    out_d = nc.dram_tensor("out", [C, DIM], F16, kind="ExternalOutput").ap()

    NCH = len(chunks)

    with tile.TileContext(nc) as tc:
        with (
            tc.tile_pool(name="persist", bufs=1) as pp,
            tc.tile_pool(name="outstage", bufs=6) as outp,
        ):
            # ---- persistent tiles ----
            xT = pp.tile([128, 2, CB, HW], F16)  # half-major
            mT = pp.tile([128, 2, CB, HW], F16)
            qT = pp.tile([128, C], F16)  # [d(2 heads), i]
            kT = pp.tile([128, C], F16)
            vTs = pp.tile([128, C], F16)
            # v natural layout + fused denominator column:
            # [j-in-block, jb, head, 64 v-cols + 1 ones-col]
            v1e = pp.tile([128, JB, HPC, DH + 1], F16)
            onesw = pp.tile([128, DH], F16)
            dummy = pp.tile([128, 512], F16)
            ident = pp.tile([128, 128], F16)
            wo_sb = pp.tile([128, DIM], F16)
            wq_sb = pp.tile([128, CB, 128], F16)
            wk_sb = pp.tile([128, CB, 128], F16)
            wv_sb = pp.tile([128, CB, 128], F16)
            jbias = pp.tile([128, JB], F32)
            ON = pp.tile([128, C], F16)  # normalized attn out^T (both heads)
            PT = pp.tile([128, NCH, JB, HPC, 512], F16)

            # ---- gpsimd: memsets FIRST so PE warmup is not stuck behind
            # the slow SWDGE triggers; SWDGE DMAs after.
            nc.gpsimd.memset(onesw[:], 1.0)
            make_identity(nc, ident[:])
            nc.vector.memset(dummy[:], 0.001)
            nc.vector.memset(v1e[:, :, :, DH : DH + 1], 1.0)

            # ---- loads ----
            # x/m half-major [128, half, cb, 512]; pieces of 0.5MB (4 cb),
            # two HWDGE queues, ordered by earliest need.
            xr = x_d.rearrange("p (hf cb i) -> p hf cb i", hf=2, cb=CB)
            mr = m_d.rearrange("p (hf cb i) -> p hf cb i", hf=2, cb=CB)
            nc.sync.dma_start(wk_sb[:], wk_d.rearrange("p (cb d) -> p cb d", cb=CB))
            nc.scalar.dma_start(wq_sb[:], wq_d.rearrange("p (cb d) -> p cb d", cb=CB))
            nc.sync.dma_start(xT[:, 0, 0:4], xr[:, 0, 0:4])
            nc.scalar.dma_start(xT[:, 0, 4:8], xr[:, 0, 4:8])
            nc.sync.dma_start(mT[:, 0, 0:4], mr[:, 0, 0:4])
            nc.scalar.dma_start(mT[:, 0, 4:8], mr[:, 0, 4:8])
            nc.sync.dma_start(mT[:, 1, 0:4], mr[:, 1, 0:4])
            nc.scalar.dma_start(mT[:, 1, 4:8], mr[:, 1, 4:8])
            nc.sync.dma_start(xT[:, 1, 0:4], xr[:, 1, 0:4])
            nc.scalar.dma_start(xT[:, 1, 4:8], xr[:, 1, 4:8])
            # SWDGE (slow ~50GB/s): small/late-needed tensors only
            nc.gpsimd.dma_start(jbias[:], jb_d)
            nc.gpsimd.dma_start(wv_sb[:], wv_d.rearrange("p (cb d) -> p cb d", cb=CB))
            nc.gpsimd.dma_start(wo_sb[:], wo_d)

            # ---------- helpers ----------
            def s_pair(ci, i0, cw, jb, sps):
                for h in range(HPC):
                    nc.tensor.matmul(
                        sps[:, h, :cw],
                        kT[h * DH : (h + 1) * DH, jb * 128 : (jb + 1) * 128],
                        qT[h * DH : (h + 1) * DH, i0 : i0 + cw],
                        start=True,
                        stop=True,
                    )
                with nc.allow_low_precision(reason="softmax weights fp16"):
                    nc.scalar.activation(
                        PT[:, ci, jb, :, :cw],
                        sps[:, :, :cw],
                        EXP,
                        bias=jbias[:, jb : jb + 1],
                        scale=scale,
                    )

            def pv_pair(ci, jb, opsT):
                cw = chunks[ci][1]
                # 65-row output per head: rows 0-63 = P@V, row 64 = denom
                for h in range(HPC):
                    nc.tensor.matmul(
                        opsT[0 : DH + 1, h, :cw],
                        v1e[:, jb, h],
                        PT[:, ci, jb, h, :cw],
                        start=(jb == 0),
                        stop=(jb == JB - 1),
                    )

            def finish_chunk(ci, opsT, den32, rec32, recd_sb):
                i0, cw = chunks[ci]
                # copy both heads' denominator rows PSUM->SBUF (the custom
                # recip op misreads PSUM at a partition offset), then recip
                nc.vector.tensor_copy(den32[0:1, :, :cw], opsT[DH : DH + 1, :, :cw])
                nc.vector.reciprocal_approx_fast(
                    rec32[0:1, :, :cw], den32[0:1, :, :cw]
                )
                nc.gpsimd.partition_broadcast(
                    recd_sb[:, :, :cw], rec32[0:1, :, :cw]
                )
                with nc.allow_low_precision(reason="attn out fp16"):
                    for h in range(HPC):
                        nc.vector.tensor_mul(
                            ON[h * DH : (h + 1) * DH, i0 : i0 + cw],
                            opsT[0:DH, h, :cw],
                            recd_sb[h * DH : (h + 1) * DH, h, :cw],
                        )

            def wo_isub(isub, psE, evicts, st_engs):
                ob = outp.tile([128, DIM], F16, tag="ob")
                for eb in range(DIM // 512):
                    dp = psE.tile([128, 512], F32, tag="dout")
                    nc.tensor.matmul(
                        dp[:],
                        ON[:, isub * 128 : (isub + 1) * 128],
                        wo_sb[:, eb * 512 : (eb + 1) * 512],
                        start=True,
                        stop=True,
                    )
                    with nc.allow_low_precision(reason="partial out fp16"):
                        evicts[eb % len(evicts)](
                            ob[:, eb * 512 : (eb + 1) * 512], dp[:]
                        )
                    # during attention all stores ride the sync queue (the
                    # scalar engine must never stall exps on DMA triggers);
                    # in the tail (after the last exp) they pair up
                    st_engs[eb % len(st_engs)].dma_start(
                        out_d[isub * 128 : (isub + 1) * 128, eb * 512 : (eb + 1) * 512],
                        ob[:, eb * 512 : (eb + 1) * 512],
                    )

            with (
                tc.tile_pool(name="psS", bufs=2, space="PSUM") as psS,
                tc.tile_pool(name="nrm", bufs=2) as nrm,
            ):
                slist = [(ci, i0, cw, jb) for ci, (i0, cw) in enumerate(chunks)
                         for jb in range(JB)]
                si = 0

                def issue_s():
                    nonlocal si
                    ci, i0, cw, jb = slist[si]
                    sps = psS.tile([128, HPC, 512], F32, tag="S")
                    s_pair(ci, i0, cw, jb, sps)
                    si += 1

                with (
                    tc.tile_pool(name="psP", bufs=2, space="PSUM") as psP,
                    tc.tile_pool(name="psQ", bufs=1, space="PSUM") as psQ,
                    tc.tile_pool(name="psT", bufs=1, space="PSUM") as psT,
                ):
                    # warm up the PE clock while loads stream
                    dmt = psQ.tile([128, 512], F32, tag="projq", name="dummy_ps")
                    for t in range(12):
                        nc.tensor.matmul(
                            dmt[0:DH, :], onesw[:], dummy[:],
                            start=(t == 0), stop=(t == 11),
                        )

                    def fill(n):
                        # dummy matmuls to keep the PE busy (and the HAM
                        # clock at 8/8) across known DMA-wait gaps
                        for t in range(n):
                            nc.tensor.matmul(
                                dmt[0:DH, :], onesw[:], dummy[:],
                                start=(t == 0), stop=(t == n - 1),
                            )

                    pps = {}

                    def _proj_half(hf, w_sb, dst, nm, cb0, cb1):
                        key = (nm, hf)
                        if key not in pps:
                            pps[key] = psP.tile([128, HW], F32, tag="projkv",
                                                name=f"p{nm}{hf}")
                        pq_ = pps[key]
                        for cb in range(cb0, cb1):
                            nc.tensor.matmul(
                                pq_[:],
                                w_sb[:, cb, :],
                                mT[:, hf, cb, :],
                                start=(cb == 0),
                                stop=(cb == CB - 1),
                            )
                        if cb1 == CB:
                            nc.vector.tensor_copy(
                                dst[:, hf * HW : (hf + 1) * HW], pq_[:]
                            )
                            del pps[key]

                    def k_half(hf, cb0=0, cb1=CB):
                        _proj_half(hf, wk_sb, kT, "k", cb0, cb1)

                    def v_half(hf, cb0=0, cb1=CB):
                        _proj_half(hf, wv_sb, vTs, "v", cb0, cb1)

                    ptt = psT.tile([128, 2, 128], F16, tag="vt")

                    def t_quarter(q):
                        for k, jb in enumerate((2 * q, 2 * q + 1)):
                            nc.tensor.transpose(
                                ptt[:, k, :], vTs[:, jb * 128 : (jb + 1) * 128],
                                ident[:],
                            )
                        nc.vector.tensor_copy(
                            v1e[:, 2 * q : 2 * q + 2, :, 0:DH],
                            ptt[:].rearrange("p a (h d) -> p a h d", h=HPC),
                        )

                    qps = {}

                    def q_chunk(ci, cb0, cb1):
                        i0, cw = chunks[ci]
                        hf = i0 // HW
                        o0 = i0 - hf * HW
                        if ci not in qps:
                            qps[ci] = psQ.tile([128, 512], F32, tag="projq",
                                               name=f"pq{ci}")
                        pq_ = qps[ci]
                        for cb in range(cb0, cb1):
                            nc.tensor.matmul(
                                pq_[:, :cw],
                                wq_sb[:, cb, :],
                                xT[:, hf, cb, o0 : o0 + cw],
                                start=(cb == 0),
                                stop=(cb == CB - 1),
                            )
                        if cb1 == CB:
                            nc.vector.tensor_copy(qT[:, i0 : i0 + cw], pq_[:, :cw])

                    # ---- projection phase; S-issues spread so the in-order
                    # PE never camps long on a blocked instruction, dummy
                    # fills sized to the known DMA arrival gaps
                    fill(6)
                    q_chunk(0, 0, 4)   # x0a (sync)
                    q_chunk(0, 4, 8)   # x0b (scalar)
                    fill(8)
                    k_half(0, 0, 4)    # m0a
                    k_half(0, 4, 8)    # m0b
                    issue_s()   # S[0] c0 jb0
                    issue_s()   # S[1]
                    v_half(0)          # wv (SWDGE) + m0
                    issue_s()   # S[2]
                    t_quarter(0)
                    issue_s()   # S[3]
                    t_quarter(1)
                    k_half(1, 0, 4)    # m1a
                    k_half(1, 4, 8)    # m1b
                    issue_s()   # S[4]
                    issue_s()   # S[5]
                    v_half(1)
                    issue_s()   # S[6]
                    t_quarter(2)
                    issue_s()   # S[7]
                    t_quarter(3)
                    q_chunk(1, 0, 8)   # x half 1
                    q_chunk(2, 0, 8)

                with (
                    tc.tile_pool(name="psO", bufs=1, space="PSUM") as psO,
                    tc.tile_pool(name="psE", bufs=2, space="PSUM") as psE,
                ):
                    opsT = psO.tile([128, HPC, 512], F32, tag="O")
                    den32 = nrm.tile([1, HPC, 512], F32, tag="den")
                    rec32 = nrm.tile([1, HPC, 512], F32, tag="rec")
                    recd_sb = nrm.tile([128, HPC, 512], F32, tag="recd")

                    vcp = nc.vector.tensor_copy
                    scp = nc.scalar.copy
                    plan = [
                        ("s",),                       # S[8] = c1 jb0
                        ("pv", 0, 0), ("pv", 0, 1),
                        ("s",),                       # S[9]
                        ("pv", 0, 2), ("pv", 0, 3),
                        ("s",),                       # S[10]
                        ("pv", 0, 4), ("pv", 0, 5),
                        ("s",),                       # S[11]
                        ("pv", 0, 6), ("pv", 0, 7),
                        ("s",),                       # S[12]
                        ("fin", 0), ("wo", 0, None), ("pv", 1, 0),
                        ("s",),                       # S[13]
                        ("wo", 1, None), ("pv", 1, 1),
                        ("s",),                       # S[14]
                        ("wo", 2, None), ("pv", 1, 2),
                        ("s",),                       # S[15]
                        ("wo", 3, None), ("pv", 1, 3),
                        ("s",),                       # S[16] = c2 jb0
                        ("pv", 1, 4),
                        ("s",),                       # S[17]
                        ("pv", 1, 5),
                        ("s",),                       # S[18]
                        ("pv", 1, 6),
                        ("s",),                       # S[19]
                        ("pv", 1, 7),
                        ("fin", 1),
                        ("s",),                       # S[20]
                        ("s",),                       # S[21]
                        ("pv", 2, 0),
                        ("s",),                       # S[22]
                        ("pv", 2, 1),
                        ("s",),                       # S[23]
                        ("pv", 2, 2), ("pv", 2, 3), ("pv", 2, 4),
                        ("pv", 2, 5), ("pv", 2, 6), ("pv", 2, 7),
                        ("wo", 4, None), ("wo", 5, "both"),
                        ("fin", 2),
                        ("wo", 6, "both"), ("wo", 7, "both"),
                    ]
                    for step in plan:
                        if step[0] == "s":
                            issue_s()
                        elif step[0] == "pv":
                            pv_pair(step[1], step[2], opsT)
                        elif step[0] == "fin":
                            finish_chunk(step[1], opsT, den32, rec32, recd_sb)
                        elif step[0] == "wo":
                            if step[2] == "both":
                                wo_isub(step[1], psE, [vcp, scp],
                                        [nc.sync, nc.scalar])
                            else:
                                wo_isub(step[1], psE, [vcp], [nc.sync])

    nc.compile()
    return nc


def _get_program(C, JB, chunks):
    key = (C, JB, tuple(chunks))
    if key not in _cache:
        _cache[key] = _build(C, JB, chunks)
    return _cache[key]


def _swizzle_w(a):  # [DIM, X] -> [128, CB*X] partition-major
    X = a.shape[1]
    return np.ascontiguousarray(
        a.reshape(CB, 128, X).transpose(1, 0, 2).reshape(128, CB * X)
    ).astype(np.float16)


def _swizzle_h(a_t, C):  # [DIM, C] -> [128, 2*CB*(C//2)] half-major
    HW = C // 2
    return np.ascontiguousarray(
        a_t.reshape(CB, 128, 2, HW).transpose(1, 2, 0, 3).reshape(128, CB * C)
    ).astype(np.float16)


def kernel(x, m, mask, Wq, Wk, Wv, Wo, bo, _trace=False, _bass_results=None):
    from concourse.bass_utils import run_bass_kernel_spmd

    x = np.asarray(x)
    m = np.asarray(m)
    mask = np.asarray(mask)
    Wq, Wk, Wv, Wo, bo = (np.asarray(a, np.float32) for a in (Wq, Wk, Wv, Wo, bo))
    b, n, dim = x.shape
    assert (b, dim) == (1, DIM)

    pm = np.concatenate([np.array([True]), mask[0]])  # [n]
    sel = np.nonzero(pm)[0]
    C0 = len(sel)
    C = max(((C0 + 127) // 128) * 128, 1024)
    JB = C // 128
    assert C == 1024, f"schedule is specialized for C=1024, got C0={C0}"
    chunks = [(0, 512), (512, 384), (896, 128)]

    x_c = np.zeros((C, DIM), np.float32)
    x_c[:C0] = x[0][sel]
    m_c = np.zeros((C, DIM), np.float32)
    m_c[:C0] = m[0][sel]
    x_t = np.ascontiguousarray(x_c.T)  # [DIM, C]
    m_t = np.ascontiguousarray(m_c.T)

    x_sw = _swizzle_h(x_t, C)
    m_sw = _swizzle_h(m_t, C)

    jbias = np.zeros(C, np.float32)
    jbias[C0:] = -1e30
    jbias_t = np.ascontiguousarray(jbias.reshape(JB, 128).T)  # [128, JB]

    nc = _get_program(C, JB, chunks)

    in_maps = []
    for c in range(N_CORES):
        h0 = c * HPC * DH  # 128*c
        in_maps.append(
            {
                "x16": x_sw,
                "m16": m_sw,
                "wq": _swizzle_w(np.ascontiguousarray(Wq[:, h0 : h0 + 128])),
                "wk": _swizzle_w(np.ascontiguousarray(Wk[:, h0 : h0 + 128])),
                "wv": _swizzle_w(np.ascontiguousarray(Wv[:, h0 : h0 + 128])),
                "wo": np.ascontiguousarray(Wo[h0 : h0 + 128, :]).astype(np.float16),
                "jbias": jbias_t,
            }
        )

    res = run_bass_kernel_spmd(
        nc, in_maps, core_ids=list(range(N_CORES)), trace=_trace
    )
    if _bass_results is not None:
        _bass_results.append(res)

    acc = np.sum(
        np.stack([np.asarray(r["out"])[:C0].astype(np.float64) for r in res.results]),
        axis=0,
    )

    # host-side: masked rows get uniform attention over ALL positions
    mv = m[0].astype(np.float64).mean(axis=0)  # mean over all j of m
    mv_out = (mv @ Wv.astype(np.float64)) @ Wo.astype(np.float64)  # [dim]

    out = np.empty((n, DIM), np.float64)
    out[sel] = acc
    out[~pm] = mv_out
    out += bo.astype(np.float64)
    return out[None].astype(np.float32)


# revision 15
# speedup vs baseline: 1.0396x; 1.0081x over previous
"""Cross-attention (b=1, n=2048, dim=1024, 16 heads x 64) on 8 TRN2 NeuronCores.

Strategy (v3):
- Tensor-parallel over heads: core k computes heads (2k, 2k+1) end to end and a
  partial output projection; host sums the 8 partials (the Wo all-reduce).
- Mask compaction on host: masked ROWS get uniform attention = (mean v) @ Wo
  computed on host; masked COLUMNS excluded from the C0 unmasked positions
  (padded to a multiple of 128), roughly halving all n^2 work.
- fp16 matmul datapath, fp32 PSUM accumulation, fp16 partials summed in
  float64 on host (all fp8 placements measured 1.1-3.7% rel err in host
  emulation -- too close to the 2e-2 gate).
- Softmax denominator FUSED into the P@V stationary as a 65th all-ones
  column (removes the separate ones-matmul, -16k PE cycles).  Normalize:
  den row -> SBUF copy -> DVE reciprocal -> gpsimd partition_broadcast ->
  one DVE multiply per head.  (The custom recip op misreads PSUM at a
  partition offset, hence the copy; DVE ops may read only one PSUM operand,
  hence the broadcast to SBUF.)
- THREE i-chunks (512, 384, 128): the last chunk is small so the
  exp->pv->finish->Wo->store chain after the final exp is short.
- PE kept dense to hold the HAM clock at 8/8: 512-wide projection streams,
  S-issues spread between independent work, dummy matmuls filling known
  DMA-wait gaps (an idle PE drops to 4/8 and halves throughput for ~8us).
- Input DMA striped over both HWDGE queues in ~0.5MB pieces ordered by
  earliest need; gpsimd SWDGE carries jbias/wv/wo only, after its memsets.
- All output stores ride the sync queue per 512-col block (scalar engine is
  kept free of DMA triggers so exps and tail evictions are never delayed).
"""
import numpy as np

N_CORES = 8
HEADS = 16
DH = 64  # head dim
DIM = 1024
HPC = HEADS // N_CORES  # heads per core = 2
CB = DIM // 128  # contraction blocks for projections (8)

_cache = {}


def _build(C, JB, chunks):
    """Build + schedule the per-core Bass program for padded length C=1024."""
    import concourse.mybir as mybir
    import concourse.tile as tile
    from concourse import bacc
    from concourse.masks import make_identity

    F32 = mybir.dt.float32
    F16 = mybir.dt.float16
    EXP = mybir.ActivationFunctionType.Exp
    scale = DIM ** -0.5
    HW = C // 2  # half width (512)

    nc = bacc.Bacc("TRN2", target_bir_lowering=False, debug=False)

    x_d = nc.dram_tensor("x16", [128, CB * C], F16, kind="ExternalInput").ap()
    m_d = nc.dram_tensor("m16", [128, CB * C], F16, kind="ExternalInput").ap()
    wq_d = nc.dram_tensor("wq", [128, DIM], F16, kind="ExternalInput").ap()
    wk_d = nc.dram_tensor("wk", [128, DIM], F16, kind="ExternalInput").ap()
    wv_d = nc.dram_tensor("wv", [128, DIM], F16, kind="ExternalInput").ap()
    wo_d = nc.dram_tensor("wo", [128, DIM], F16, kind="ExternalInput").ap()
    jb_d = nc.dram_tensor("jbias", [128, JB], F32, kind="ExternalInput").ap()
    out_d = nc.dram_tensor("out", [C, DIM], F16, kind="ExternalOutput").ap()

    NCH = len(chunks)

    with tile.TileContext(nc) as tc:
        with (
            tc.tile_pool(name="persist", bufs=1) as pp,
            tc.tile_pool(name="outstage", bufs=6) as outp,
        ):
            # ---- persistent tiles ----
            xT = pp.tile([128, 2, CB, HW], F16)  # half-major
            mT = pp.tile([128, 2, CB, HW], F16)
            qT = pp.tile([128, C], F16)  # [d(2 heads), i]
            kT = pp.tile([128, C], F16)
            vTs = pp.tile([128, C], F16)
            # v natural layout + fused denominator column:
            # [j-in-block, jb, head, 64 v-cols + 1 ones-col]
            v1e = pp.tile([128, JB, HPC, DH + 1], F16)
            onesw = pp.tile([128, DH], F16)
            dummy = pp.tile([128, 512], F16)
            ident = pp.tile([128, 128], F16)
            wo_sb = pp.tile([128, DIM], F16)
            wq_sb = pp.tile([128, CB, 128], F16)
            wk_sb = pp.tile([128, CB, 128], F16)
            wv_sb = pp.tile([128, CB, 128], F16)
            jbias = pp.tile([128, JB], F32)
            ON = pp.tile([128, C], F16)  # normalized attn out^T (both heads)
            PT = pp.tile([128, NCH, JB, HPC, 512], F16)

            # ---- gpsimd: memsets FIRST so PE warmup is not stuck behind
            # the slow SWDGE triggers; SWDGE DMAs after.
            nc.gpsimd.memset(onesw[:], 1.0)
            make_identity(nc, ident[:])
            nc.vector.memset(dummy[:], 0.001)
            nc.vector.memset(v1e[:, :, :, DH : DH + 1], 1.0)

            # ---- loads ----
            # x/m half-major [128, half, cb, 512]; pieces of 0.5MB (4 cb),
            # two HWDGE queues, ordered by earliest need.
            xr = x_d.rearrange("p (hf cb i) -> p hf cb i", hf=2, cb=CB)
            mr = m_d.rearrange("p (hf cb i) -> p hf cb i", hf=2, cb=CB)
            nc.sync.dma_start(wk_sb[:], wk_d.rearrange("p (cb d) -> p cb d", cb=CB))
            nc.scalar.dma_start(wq_sb[:], wq_d.rearrange("p (cb d) -> p cb d", cb=CB))
            nc.sync.dma_start(xT[:, 0, 0:4], xr[:, 0, 0:4])
            nc.scalar.dma_start(xT[:, 0, 4:8], xr[:, 0, 4:8])
            nc.sync.dma_start(mT[:, 0, 0:4], mr[:, 0, 0:4])
            nc.scalar.dma_start(mT[:, 0, 4:8], mr[:, 0, 4:8])
            nc.sync.dma_start(mT[:, 1, 0:4], mr[:, 1, 0:4])
            nc.scalar.dma_start(mT[:, 1, 4:8], mr[:, 1, 4:8])
            nc.sync.dma_start(xT[:, 1, 0:4], xr[:, 1, 0:4])
            nc.scalar.dma_start(xT[:, 1, 4:8], xr[:, 1, 4:8])
            # SWDGE (slow ~50GB/s): small/late-needed tensors only
            nc.gpsimd.dma_start(jbias[:], jb_d)
            nc.gpsimd.dma_start(wv_sb[:], wv_d.rearrange("p (cb d) -> p cb d", cb=CB))
            nc.gpsimd.dma_start(wo_sb[:], wo_d)

            # ---------- helpers ----------
            def s_pair(ci, i0, cw, jb, sps):
                for h in range(HPC):
                    nc.tensor.matmul(
                        sps[:, h, :cw],
                        kT[h * DH : (h + 1) * DH, jb * 128 : (jb + 1) * 128],
                        qT[h * DH : (h + 1) * DH, i0 : i0 + cw],
                        start=True,
                        stop=True,
                    )
                with nc.allow_low_precision(reason="softmax weights fp16"):
                    nc.scalar.activation(
                        PT[:, ci, jb, :, :cw],
                        sps[:, :, :cw],
                        EXP,
                        bias=jbias[:, jb : jb + 1],
                        scale=scale,
                    )

            def pv_pair(ci, jb, opsT):
                cw = chunks[ci][1]
                # 65-row output per head: rows 0-63 = P@V, row 64 = denom
                for h in range(HPC):
                    nc.tensor.matmul(
                        opsT[0 : DH + 1, h, :cw],
                        v1e[:, jb, h],
                        PT[:, ci, jb, h, :cw],
                        start=(jb == 0),
                        stop=(jb == JB - 1),
                    )

            def finish_chunk(ci, opsT, den32, rec32, recd_sb):
                i0, cw = chunks[ci]
                # copy both heads' denominator rows PSUM->SBUF (the custom
                # recip op misreads PSUM at a partition offset), then recip
                nc.vector.tensor_copy(den32[0:1, :, :cw], opsT[DH : DH + 1, :, :cw])
                nc.vector.reciprocal_approx_fast(
                    rec32[0:1, :, :cw], den32[0:1, :, :cw]
                )
                nc.gpsimd.partition_broadcast(
                    recd_sb[:, :, :cw], rec32[0:1, :, :cw]
                )
                with nc.allow_low_precision(reason="attn out fp16"):
                    for h in range(HPC):
                        nc.vector.tensor_mul(
                            ON[h * DH : (h + 1) * DH, i0 : i0 + cw],
                            opsT[0:DH, h, :cw],
                            recd_sb[h * DH : (h + 1) * DH, h, :cw],
                        )

            def wo_isub(isub, psE, evicts, st_engs):
                ob = outp.tile([128, DIM], F16, tag="ob")
                for eb in range(DIM // 512):
                    dp = psE.tile([128, 512], F32, tag="dout")
                    nc.tensor.matmul(
                        dp[:],
                        ON[:, isub * 128 : (isub + 1) * 128],
                        wo_sb[:, eb * 512 : (eb + 1) * 512],
                        start=True,
                        stop=True,
                    )
                    with nc.allow_low_precision(reason="partial out fp16"):
                        evicts[eb % len(evicts)](
                            ob[:, eb * 512 : (eb + 1) * 512], dp[:]
                        )
                    # during attention all stores ride the sync queue (the
                    # scalar engine must never stall exps on DMA triggers);
                    # in the tail (after the last exp) they pair up
                    st_engs[eb % len(st_engs)].dma_start(
                        out_d[isub * 128 : (isub + 1) * 128, eb * 512 : (eb + 1) * 512],
                        ob[:, eb * 512 : (eb + 1) * 512],
                    )

            with (
                tc.tile_pool(name="psS", bufs=2, space="PSUM") as psS,
                tc.tile_pool(name="nrm", bufs=2) as nrm,
            ):
                slist = [(ci, i0, cw, jb) for ci, (i0, cw) in enumerate(chunks)
                         for jb in range(JB)]
                si = 0

                def issue_s():
                    nonlocal si
                    ci, i0, cw, jb = slist[si]
                    sps = psS.tile([128, HPC, 512], F32, tag="S")
                    s_pair(ci, i0, cw, jb, sps)
                    si += 1

                with (
                    tc.tile_pool(name="psP", bufs=2, space="PSUM") as psP,
                    tc.tile_pool(name="psQ", bufs=1, space="PSUM") as psQ,
                    tc.tile_pool(name="psT", bufs=1, space="PSUM") as psT,
                ):
                    # warm up the PE clock while loads stream
                    dmt = psQ.tile([128, 512], F32, tag="projq", name="dummy_ps")
                    for t in range(12):
                        nc.tensor.matmul(
                            dmt[0:DH, :], onesw[:], dummy[:],
                            start=(t == 0), stop=(t == 11),
                        )

                    def fill(n):
                        # dummy matmuls to keep the PE busy (and the HAM
                        # clock at 8/8) across known DMA-wait gaps
                        for t in range(n):
                            nc.tensor.matmul(
                                dmt[0:DH, :], onesw[:], dummy[:],
                                start=(t == 0), stop=(t == n - 1),
                            )

                    pps = {}

                    def _proj_half(hf, w_sb, dst, nm, cb0, cb1):
                        key = (nm, hf)
                        if key not in pps:
                            pps[key] = psP.tile([128, HW], F32, tag="projkv",
                                                name=f"p{nm}{hf}")
                        pq_ = pps[key]
                        for cb in range(cb0, cb1):
                            nc.tensor.matmul(
                                pq_[:],
                                w_sb[:, cb, :],
                                mT[:, hf, cb, :],
                                start=(cb == 0),
                                stop=(cb == CB - 1),
                            )
                        if cb1 == CB:
                            nc.vector.tensor_copy(
                                dst[:, hf * HW : (hf + 1) * HW], pq_[:]
                            )
                            del pps[key]

                    def k_half(hf, cb0=0, cb1=CB):
                        _proj_half(hf, wk_sb, kT, "k", cb0, cb1)

                    def v_half(hf, cb0=0, cb1=CB):
                        _proj_half(hf, wv_sb, vTs, "v", cb0, cb1)

                    ptt = psT.tile([128, 2, 128], F16, tag="vt")

                    def t_quarter(q):
                        for k, jb in enumerate((2 * q, 2 * q + 1)):
                            nc.tensor.transpose(
                                ptt[:, k, :], vTs[:, jb * 128 : (jb + 1) * 128],
                                ident[:],
                            )
                        nc.vector.tensor_copy(
                            v1e[:, 2 * q : 2 * q + 2, :, 0:DH],
                            ptt[:].rearrange("p a (h d) -> p a h d", h=HPC),
                        )

                    qps = {}

                    def q_chunk(ci, cb0, cb1):
                        i0, cw = chunks[ci]
                        hf = i0 // HW
                        o0 = i0 - hf * HW
                        if ci not in qps:
                            qps[ci] = psQ.tile([128, 512], F32, tag="projq",
                                               name=f"pq{ci}")
                        pq_ = qps[ci]
                        for cb in range(cb0, cb1):
                            nc.tensor.matmul(
                                pq_[:, :cw],
                                wq_sb[:, cb, :],
                                xT[:, hf, cb, o0 : o0 + cw],
                                start=(cb == 0),
                                stop=(cb == CB - 1),
                            )
                        if cb1 == CB:
                            nc.vector.tensor_copy(qT[:, i0 : i0 + cw], pq_[:, :cw])

                    # ---- projection phase; S-issues spread so the in-order
                    # PE never camps long on a blocked instruction, dummy
                    # fills sized to the known DMA arrival gaps
                    fill(4)
                    q_chunk(0, 0, 4)   # x0a (sync)
                    q_chunk(0, 4, 8)   # x0b (scalar)
                    fill(24)
                    k_half(0, 0, 4)    # m0a
                    k_half(0, 4, 8)    # m0b
                    issue_s()   # S[0] c0 jb0
                    issue_s()   # S[1]
                    v_half(0)          # wv (SWDGE) + m0
                    issue_s()   # S[2]
                    t_quarter(0)
                    issue_s()   # S[3]
                    t_quarter(1)
                    k_half(1, 0, 4)    # m1a
                    k_half(1, 4, 8)    # m1b
                    issue_s()   # S[4]
                    issue_s()   # S[5]
                    v_half(1)
                    issue_s()   # S[6]
                    t_quarter(2)
                    issue_s()   # S[7]
                    t_quarter(3)
                    q_chunk(1, 0, 8)   # x half 1
                    q_chunk(2, 0, 8)

                with (
                    tc.tile_pool(name="psO", bufs=1, space="PSUM") as psO,
                    tc.tile_pool(name="psE", bufs=2, space="PSUM") as psE,
                ):
                    opsT = psO.tile([128, HPC, 512], F32, tag="O")
                    den32 = nrm.tile([1, HPC, 512], F32, tag="den")
                    rec32 = nrm.tile([1, HPC, 512], F32, tag="rec")
                    recd_sb = nrm.tile([128, HPC, 512], F32, tag="recd")

                    vcp = nc.vector.tensor_copy
                    scp = nc.scalar.copy
                    plan = [
                        ("s",),                       # S[8] = c1 jb0
                        ("pv", 0, 0),
                        ("s",),                       # S[9]
                        ("pv", 0, 1), ("pv", 0, 2),
                        ("s",),                       # S[10]
                        ("pv", 0, 3), ("pv", 0, 4),
                        ("s",),                       # S[11]
                        ("pv", 0, 5), ("pv", 0, 6),
                        ("s",),                       # S[12]
                        ("pv", 0, 7), ("fin", 0),
                        ("s",),                       # S[13]
                        ("pv", 1, 0),
                        ("s",),                       # S[14]
                        ("pv", 1, 1),
                        ("s",),                       # S[15]
                        ("pv", 1, 2),
                        ("s",),                       # S[16] = c2 jb0
                        ("pv", 1, 3),
                        ("s",),                       # S[17]
                        ("pv", 1, 4), ("wo", 0, None),
                        ("s",),                       # S[18]
                        ("pv", 1, 5), ("wo", 1, None),
                        ("s",),                       # S[19]
                        ("pv", 1, 6),
                        ("s",),                       # S[20]
                        ("pv", 1, 7), ("fin", 1),
                        ("s",),                       # S[21]
                        ("pv", 2, 0),
                        ("s",),                       # S[22]
                        ("pv", 2, 1),
                        ("s",),                       # S[23]
                        ("pv", 2, 2), ("pv", 2, 3), ("pv", 2, 4),
                        ("pv", 2, 5), ("pv", 2, 6), ("pv", 2, 7),
                        ("fin", 2),
                        ("wo", 2, "both"), ("wo", 3, "both"),
                        ("wo", 4, "both"), ("wo", 5, "both"),
                        ("wo", 6, "both"), ("wo", 7, "both"),
                    ]
                    for step in plan:
                        if step[0] == "s":
                            issue_s()
                        elif step[0] == "pv":
                            pv_pair(step[1], step[2], opsT)
                        elif step[0] == "fin":
                            finish_chunk(step[1], opsT, den32, rec32, recd_sb)
                        elif step[0] == "wo":
                            if step[2] == "both":
                                wo_isub(step[1], psE, [vcp, scp],
                                        [nc.sync, nc.scalar])
                            else:
                                wo_isub(step[1], psE, [vcp], [nc.sync])

    nc.compile()
    return nc


def _get_program(C, JB, chunks):
    key = (C, JB, tuple(chunks))
    if key not in _cache:
        _cache[key] = _build(C, JB, chunks)
    return _cache[key]


def _swizzle_w(a):  # [DIM, X] -> [128, CB*X] partition-major
    X = a.shape[1]
    return np.ascontiguousarray(
        a.reshape(CB, 128, X).transpose(1, 0, 2).reshape(128, CB * X)
    ).astype(np.float16)


def _swizzle_h(a_t, C):  # [DIM, C] -> [128, 2*CB*(C//2)] half-major
    HW = C // 2
    return np.ascontiguousarray(
        a_t.reshape(CB, 128, 2, HW).transpose(1, 2, 0, 3).reshape(128, CB * C)
    ).astype(np.float16)


def kernel(x, m, mask, Wq, Wk, Wv, Wo, bo, _trace=False, _bass_results=None):
    from concourse.bass_utils import run_bass_kernel_spmd

    x = np.asarray(x)
    m = np.asarray(m)
    mask = np.asarray(mask)
    Wq, Wk, Wv, Wo, bo = (np.asarray(a, np.float32) for a in (Wq, Wk, Wv, Wo, bo))
    b, n, dim = x.shape
    assert (b, dim) == (1, DIM)

    pm = np.concatenate([np.array([True]), mask[0]])  # [n]
    sel = np.nonzero(pm)[0]
    C0 = len(sel)
    C = max(((C0 + 127) // 128) * 128, 1024)
    JB = C // 128
    assert C == 1024, f"schedule is specialized for C=1024, got C0={C0}"
    chunks = [(0, 512), (512, 384), (896, 128)]

    x_c = np.zeros((C, DIM), np.float32)
    x_c[:C0] = x[0][sel]
    m_c = np.zeros((C, DIM), np.float32)
    m_c[:C0] = m[0][sel]
    x_t = np.ascontiguousarray(x_c.T)  # [DIM, C]
    m_t = np.ascontiguousarray(m_c.T)

    x_sw = _swizzle_h(x_t, C)
    m_sw = _swizzle_h(m_t, C)

    jbias = np.zeros(C, np.float32)
    jbias[C0:] = -1e30
    jbias_t = np.ascontiguousarray(jbias.reshape(JB, 128).T)  # [128, JB]

    nc = _get_program(C, JB, chunks)

    in_maps = []
    for c in range(N_CORES):
        h0 = c * HPC * DH  # 128*c
        in_maps.append(
            {
                "x16": x_sw,
                "m16": m_sw,
                "wq": _swizzle_w(np.ascontiguousarray(Wq[:, h0 : h0 + 128])),
                "wk": _swizzle_w(np.ascontiguousarray(Wk[:, h0 : h0 + 128])),
                "wv": _swizzle_w(np.ascontiguousarray(Wv[:, h0 : h0 + 128])),
                "wo": np.ascontiguousarray(Wo[h0 : h0 + 128, :]).astype(np.float16),
                "jbias": jbias_t,
            }
        )

    res = run_bass_kernel_spmd(
        nc, in_maps, core_ids=list(range(N_CORES)), trace=_trace
    )
    if _bass_results is not None:
        _bass_results.append(res)

    acc = np.sum(
        np.stack([np.asarray(r["out"])[:C0].astype(np.float64) for r in res.results]),
        axis=0,
    )

    # host-side: masked rows get uniform attention over ALL positions
    mv = m[0].astype(np.float64).mean(axis=0)  # mean over all j of m
    mv_out = (mv @ Wv.astype(np.float64)) @ Wo.astype(np.float64)  # [dim]

    out = np.empty((n, DIM), np.float64)
    out[sel] = acc
    out[~pm] = mv_out
    out += bo.astype(np.float64)
    return out[None].astype(np.float32)


# revision 16
# speedup vs baseline: 1.0926x; 1.0510x over previous
"""Cross-attention (b=1, n=2048, dim=1024, 16 heads x 64) on 8 TRN2 NeuronCores.

Strategy (v3):
- Tensor-parallel over heads: core k computes heads (2k, 2k+1) end to end and a
  partial output projection; host sums the 8 partials (the Wo all-reduce).
- Mask compaction on host: masked ROWS get uniform attention = (mean v) @ Wo
  computed on host; masked COLUMNS excluded from the C0 unmasked positions
  (padded to a multiple of 128), roughly halving all n^2 work.
- fp16 matmul datapath, fp32 PSUM accumulation, fp16 partials summed in
  float64 on host (all fp8 placements measured 1.1-3.7% rel err in host
  emulation -- too close to the 2e-2 gate).
- Softmax denominator FUSED into the P@V stationary as a 65th all-ones
  column (removes the separate ones-matmul, -16k PE cycles).  Normalize:
  den row -> SBUF copy -> DVE reciprocal -> gpsimd partition_broadcast ->
  one DVE multiply per head.  (The custom recip op misreads PSUM at a
  partition offset, hence the copy; DVE ops may read only one PSUM operand,
  hence the broadcast to SBUF.)
- THREE i-chunks (512, 384, 128): the last chunk is small so the
  exp->pv->finish->Wo->store chain after the final exp is short.
- PE kept dense to hold the HAM clock at 8/8: 512-wide projection streams,
  S-issues spread between independent work, dummy matmuls filling known
  DMA-wait gaps (an idle PE drops to 4/8 and halves throughput for ~8us).
- Input DMA striped over both HWDGE queues in ~0.5MB pieces ordered by
  earliest need; gpsimd SWDGE carries jbias/wv/wo only, after its memsets.
- All output stores ride the sync queue per 512-col block (scalar engine is
  kept free of DMA triggers so exps and tail evictions are never delayed).
"""
import numpy as np

N_CORES = 8
HEADS = 16
DH = 64  # head dim
DIM = 1024
HPC = HEADS // N_CORES  # heads per core = 2
CB = DIM // 128  # contraction blocks for projections (8)

_cache = {}


def _build(C, JB, chunks):
    """Build + schedule the per-core Bass program for padded length C=1024."""
    import concourse.mybir as mybir
    import concourse.tile as tile
    from concourse import bacc
    from concourse.masks import make_identity

    F32 = mybir.dt.float32
    F16 = mybir.dt.float16
    EXP = mybir.ActivationFunctionType.Exp
    scale = DIM ** -0.5
    HW = C // 2  # half width (512)

    nc = bacc.Bacc("TRN2", target_bir_lowering=False, debug=False)

    x_d = nc.dram_tensor("x16", [128, CB * C], F16, kind="ExternalInput").ap()
    m_d = nc.dram_tensor("m16", [128, CB * C], F16, kind="ExternalInput").ap()
    wq_d = nc.dram_tensor("wq", [128, DIM], F16, kind="ExternalInput").ap()
    wk_d = nc.dram_tensor("wk", [128, DIM], F16, kind="ExternalInput").ap()
    wv_d = nc.dram_tensor("wv", [128, DIM], F16, kind="ExternalInput").ap()
    wo_d = nc.dram_tensor("wo", [128, DIM], F16, kind="ExternalInput").ap()
    jb_d = nc.dram_tensor("jbias", [128, JB], F32, kind="ExternalInput").ap()
    out_d = nc.dram_tensor("out", [C, DIM], F16, kind="ExternalOutput").ap()

    NCH = len(chunks)

    with tile.TileContext(nc) as tc:
        with (
            tc.tile_pool(name="persist", bufs=1) as pp,
            tc.tile_pool(name="outstage", bufs=6) as outp,
        ):
            # ---- persistent tiles ----
            xT = pp.tile([128, 2, CB, HW], F16)  # half-major
            mT = pp.tile([128, 2, CB, HW], F16)
            qT = pp.tile([128, C], F16)  # [d(2 heads), i]
            kT = pp.tile([128, C], F16)
            vTs = pp.tile([128, C], F16)
            # v natural layout + fused denominator column:
            # [j-in-block, jb, head, 64 v-cols + 1 ones-col]
            v1e = pp.tile([128, JB, HPC, DH + 1], F16)
            onesw = pp.tile([128, DH], F16)
            dummy = pp.tile([128, 512], F16)
            ident = pp.tile([128, 128], F16)
            wo_sb = pp.tile([128, DIM], F16)
            wq_sb = pp.tile([128, CB, 128], F16)
            wk_sb = pp.tile([128, CB, 128], F16)
            wv_sb = pp.tile([128, CB, 128], F16)
            jbias = pp.tile([128, JB], F32)
            ON = pp.tile([128, C], F16)  # normalized attn out^T (both heads)
            PT = pp.tile([128, NCH, JB, HPC, 512], F16)

            # ---- gpsimd: memsets FIRST so PE warmup is not stuck behind
            # the slow SWDGE triggers; SWDGE DMAs after.
            nc.gpsimd.memset(onesw[:], 1.0)
            make_identity(nc, ident[:])
            nc.vector.memset(dummy[:], 0.001)
            nc.vector.memset(v1e[:, :, :, DH : DH + 1], 1.0)

            # ---- loads ----
            # x/m half-major [128, half, cb, 512]; pieces of 0.5MB (4 cb),
            # two HWDGE queues, ordered by earliest need.
            xr = x_d.rearrange("p (hf cb i) -> p hf cb i", hf=2, cb=CB)
            mr = m_d.rearrange("p (hf cb i) -> p hf cb i", hf=2, cb=CB)
            nc.sync.dma_start(wk_sb[:], wk_d.rearrange("p (cb d) -> p cb d", cb=CB))
            nc.scalar.dma_start(wq_sb[:], wq_d.rearrange("p (cb d) -> p cb d", cb=CB))
            nc.sync.dma_start(xT[:, 0, 0:4], xr[:, 0, 0:4])
            nc.scalar.dma_start(xT[:, 0, 4:8], xr[:, 0, 4:8])
            nc.sync.dma_start(mT[:, 0, 0:4], mr[:, 0, 0:4])
            nc.scalar.dma_start(mT[:, 0, 4:8], mr[:, 0, 4:8])
            nc.sync.dma_start(mT[:, 1, 0:4], mr[:, 1, 0:4])
            nc.scalar.dma_start(mT[:, 1, 4:8], mr[:, 1, 4:8])
            nc.sync.dma_start(xT[:, 1, 0:4], xr[:, 1, 0:4])
            nc.scalar.dma_start(xT[:, 1, 4:8], xr[:, 1, 4:8])
            # SWDGE (slow ~50GB/s): small/late-needed tensors only
            nc.gpsimd.dma_start(jbias[:], jb_d)
            nc.gpsimd.dma_start(wv_sb[:], wv_d.rearrange("p (cb d) -> p cb d", cb=CB))
            nc.gpsimd.dma_start(wo_sb[:], wo_d)

            # ---------- helpers ----------
            def s_pair(ci, i0, cw, jb, sps):
                for h in range(HPC):
                    nc.tensor.matmul(
                        sps[:, h, :cw],
                        kT[h * DH : (h + 1) * DH, jb * 128 : (jb + 1) * 128],
                        qT[h * DH : (h + 1) * DH, i0 : i0 + cw],
                        start=True,
                        stop=True,
                    )
                with nc.allow_low_precision(reason="softmax weights fp16"):
                    nc.scalar.activation(
                        PT[:, ci, jb, :, :cw],
                        sps[:, :, :cw],
                        EXP,
                        bias=jbias[:, jb : jb + 1],
                        scale=scale,
                    )

            def pv_pair(ci, jb, opsT):
                cw = chunks[ci][1]
                # 65-row output per head: rows 0-63 = P@V, row 64 = denom
                for h in range(HPC):
                    nc.tensor.matmul(
                        opsT[0 : DH + 1, h, :cw],
                        v1e[:, jb, h],
                        PT[:, ci, jb, h, :cw],
                        start=(jb == 0),
                        stop=(jb == JB - 1),
                    )

            def finish_chunk(ci, opsT, den32, rec32, recd_sb, onu):
                i0, cw = chunks[ci]
                # Read PSUM out FIRST (den row + unnormalized PV rows) so the
                # next chunk's pv start=True reset is unblocked quickly; the
                # slow recip -> gpsimd-broadcast -> multiply then runs purely
                # in SBUF in the shadow of the next chunk's work.
                nc.vector.tensor_copy(den32[0:1, :, :cw], opsT[DH : DH + 1, :, :cw])
                with nc.allow_low_precision(reason="attn out fp16"):
                    nc.vector.tensor_copy(onu[0:DH, :, :cw], opsT[0:DH, :, :cw])
                nc.vector.reciprocal_approx_fast(
                    rec32[0:1, :, :cw], den32[0:1, :, :cw]
                )
                nc.gpsimd.partition_broadcast(
                    recd_sb[:, :, :cw], rec32[0:1, :, :cw]
                )
                with nc.allow_low_precision(reason="attn out fp16"):
                    for h in range(HPC):
                        nc.vector.tensor_mul(
                            ON[h * DH : (h + 1) * DH, i0 : i0 + cw],
                            onu[0:DH, h, :cw],
                            recd_sb[0:DH, h, :cw],
                        )

            def wo_isub(isub, psE, evicts, st_engs):
                ob = outp.tile([128, DIM], F16, tag="ob")
                for eb in range(DIM // 512):
                    dp = psE.tile([128, 512], F32, tag="dout")
                    nc.tensor.matmul(
                        dp[:],
                        ON[:, isub * 128 : (isub + 1) * 128],
                        wo_sb[:, eb * 512 : (eb + 1) * 512],
                        start=True,
                        stop=True,
                    )
                    with nc.allow_low_precision(reason="partial out fp16"):
                        evicts[eb % len(evicts)](
                            ob[:, eb * 512 : (eb + 1) * 512], dp[:]
                        )
                    # during attention all stores ride the sync queue (the
                    # scalar engine must never stall exps on DMA triggers);
                    # in the tail (after the last exp) they pair up
                    st_engs[eb % len(st_engs)].dma_start(
                        out_d[isub * 128 : (isub + 1) * 128, eb * 512 : (eb + 1) * 512],
                        ob[:, eb * 512 : (eb + 1) * 512],
                    )

            with (
                tc.tile_pool(name="psS", bufs=2, space="PSUM") as psS,
                tc.tile_pool(name="nrm", bufs=2) as nrm,
            ):
                slist = [(ci, i0, cw, jb) for ci, (i0, cw) in enumerate(chunks)
                         for jb in range(JB)]
                si = 0

                def issue_s():
                    nonlocal si
                    ci, i0, cw, jb = slist[si]
                    sps = psS.tile([128, HPC, 512], F32, tag="S")
                    s_pair(ci, i0, cw, jb, sps)
                    si += 1

                with (
                    tc.tile_pool(name="psP", bufs=2, space="PSUM") as psP,
                    tc.tile_pool(name="psQ", bufs=1, space="PSUM") as psQ,
                    tc.tile_pool(name="psT", bufs=1, space="PSUM") as psT,
                ):
                    # warm up the PE clock while loads stream
                    dmt = psQ.tile([128, 512], F32, tag="projq", name="dummy_ps")
                    for t in range(12):
                        nc.tensor.matmul(
                            dmt[0:DH, :], onesw[:], dummy[:],
                            start=(t == 0), stop=(t == 11),
                        )

                    def fill(n):
                        # dummy matmuls to keep the PE busy (and the HAM
                        # clock at 8/8) across known DMA-wait gaps
                        for t in range(n):
                            nc.tensor.matmul(
                                dmt[0:DH, :], onesw[:], dummy[:],
                                start=(t == 0), stop=(t == n - 1),
                            )

                    pps = {}

                    def _proj_half(hf, w_sb, dst, nm, cb0, cb1):
                        key = (nm, hf)
                        if key not in pps:
                            pps[key] = psP.tile([128, HW], F32, tag="projkv",
                                                name=f"p{nm}{hf}")
                        pq_ = pps[key]
                        for cb in range(cb0, cb1):
                            nc.tensor.matmul(
                                pq_[:],
                                w_sb[:, cb, :],
                                mT[:, hf, cb, :],
                                start=(cb == 0),
                                stop=(cb == CB - 1),
                            )
                        if cb1 == CB:
                            nc.vector.tensor_copy(
                                dst[:, hf * HW : (hf + 1) * HW], pq_[:]
                            )
                            del pps[key]

                    def k_half(hf, cb0=0, cb1=CB):
                        _proj_half(hf, wk_sb, kT, "k", cb0, cb1)

                    def v_half(hf, cb0=0, cb1=CB):
                        _proj_half(hf, wv_sb, vTs, "v", cb0, cb1)

                    ptt = psT.tile([128, 2, 128], F16, tag="vt")

                    def t_quarter(q):
                        for k, jb in enumerate((2 * q, 2 * q + 1)):
                            nc.tensor.transpose(
                                ptt[:, k, :], vTs[:, jb * 128 : (jb + 1) * 128],
                                ident[:],
                            )
                        nc.vector.tensor_copy(
                            v1e[:, 2 * q : 2 * q + 2, :, 0:DH],
                            ptt[:].rearrange("p a (h d) -> p a h d", h=HPC),
                        )

                    qps = {}

                    def q_chunk(ci, cb0, cb1):
                        i0, cw = chunks[ci]
                        hf = i0 // HW
                        o0 = i0 - hf * HW
                        if ci not in qps:
                            qps[ci] = psQ.tile([128, 512], F32, tag="projq",
                                               name=f"pq{ci}")
                        pq_ = qps[ci]
                        for cb in range(cb0, cb1):
                            nc.tensor.matmul(
                                pq_[:, :cw],
                                wq_sb[:, cb, :],
                                xT[:, hf, cb, o0 : o0 + cw],
                                start=(cb == 0),
                                stop=(cb == CB - 1),
                            )
                        if cb1 == CB:
                            nc.vector.tensor_copy(qT[:, i0 : i0 + cw], pq_[:, :cw])

                    # ---- projection phase; S-issues spread so the in-order
                    # PE never camps long on a blocked instruction, dummy
                    # fills sized to the known DMA arrival gaps
                    fill(4)
                    q_chunk(0, 0, 4)   # x0a (sync)
                    q_chunk(0, 4, 8)   # x0b (scalar)
                    fill(24)
                    k_half(0, 0, 4)    # m0a
                    k_half(0, 4, 8)    # m0b
                    issue_s()   # S[0] c0 jb0
                    issue_s()   # S[1]
                    v_half(0)          # wv (SWDGE) + m0
                    issue_s()   # S[2]
                    t_quarter(0)
                    issue_s()   # S[3]
                    t_quarter(1)
                    k_half(1, 0, 4)    # m1a
                    k_half(1, 4, 8)    # m1b
                    issue_s()   # S[4]
                    issue_s()   # S[5]
                    v_half(1)
                    issue_s()   # S[6]
                    t_quarter(2)
                    issue_s()   # S[7]
                    t_quarter(3)
                    q_chunk(1, 0, 8)   # x half 1
                    q_chunk(2, 0, 8)

                with (
                    tc.tile_pool(name="psO", bufs=1, space="PSUM") as psO,
                    tc.tile_pool(name="psE", bufs=2, space="PSUM") as psE,
                ):
                    opsT = psO.tile([128, HPC, 512], F32, tag="O")
                    den32 = nrm.tile([1, HPC, 512], F32, tag="den")
                    onu = nrm.tile([128, HPC, 512], F16, tag="onu")
                    rec32 = nrm.tile([1, HPC, 512], F32, tag="rec")
                    recd_sb = nrm.tile([128, HPC, 512], F32, tag="recd")

                    vcp = nc.vector.tensor_copy
                    scp = nc.scalar.copy
                    plan = [
                        ("s",),                       # S[8] = c1 jb0
                        ("pv", 0, 0),
                        ("s",),                       # S[9]
                        ("pv", 0, 1), ("pv", 0, 2),
                        ("s",),                       # S[10]
                        ("pv", 0, 3), ("pv", 0, 4),
                        ("s",),                       # S[11]
                        ("pv", 0, 5), ("pv", 0, 6),
                        ("s",),                       # S[12]
                        ("pv", 0, 7), ("fin", 0),
                        ("s",),                       # S[13]
                        ("pv", 1, 0),
                        ("s",),                       # S[14]
                        ("pv", 1, 1),
                        ("s",),                       # S[15]
                        ("pv", 1, 2),
                        ("s",),                       # S[16] = c2 jb0
                        ("pv", 1, 3),
                        ("s",),                       # S[17]
                        ("pv", 1, 4), ("wo", 0, None),
                        ("s",),                       # S[18]
                        ("pv", 1, 5), ("wo", 1, None),
                        ("s",),                       # S[19]
                        ("pv", 1, 6),
                        ("s",),                       # S[20]
                        ("pv", 1, 7), ("fin", 1),
                        ("s",),                       # S[21]
                        ("pv", 2, 0),
                        ("s",),                       # S[22]
                        ("pv", 2, 1),
                        ("s",),                       # S[23]
                        ("pv", 2, 2), ("pv", 2, 3), ("pv", 2, 4),
                        ("pv", 2, 5), ("pv", 2, 6), ("pv", 2, 7),
                        ("fin", 2),
                        ("wo", 2, "both"), ("wo", 3, "both"),
                        ("wo", 4, "both"), ("wo", 5, "both"),
                        ("wo", 6, "both"), ("wo", 7, "both"),
                    ]
                    for step in plan:
                        if step[0] == "s":
                            issue_s()
                        elif step[0] == "pv":
                            pv_pair(step[1], step[2], opsT)
                        elif step[0] == "fin":
                            finish_chunk(step[1], opsT, den32, rec32, recd_sb, onu)
                        elif step[0] == "wo":
                            if step[2] == "both":
                                wo_isub(step[1], psE, [vcp, scp],
                                        [nc.sync, nc.scalar])
                            else:
                                wo_isub(step[1], psE, [vcp], [nc.sync])

    nc.compile()
    return nc


def _get_program(C, JB, chunks):
    key = (C, JB, tuple(chunks))
    if key not in _cache:
        _cache[key] = _build(C, JB, chunks)
    return _cache[key]


def _swizzle_w(a):  # [DIM, X] -> [128, CB*X] partition-major
    X = a.shape[1]
    return np.ascontiguousarray(
        a.reshape(CB, 128, X).transpose(1, 0, 2).reshape(128, CB * X)
    ).astype(np.float16)


def _swizzle_h(a_t, C):  # [DIM, C] -> [128, 2*CB*(C//2)] half-major
    HW = C // 2
    return np.ascontiguousarray(
        a_t.reshape(CB, 128, 2, HW).transpose(1, 2, 0, 3).reshape(128, CB * C)
    ).astype(np.float16)


def kernel(x, m, mask, Wq, Wk, Wv, Wo, bo, _trace=False, _bass_results=None):
    from concourse.bass_utils import run_bass_kernel_spmd

    x = np.asarray(x)
    m = np.asarray(m)
    mask = np.asarray(mask)
    Wq, Wk, Wv, Wo, bo = (np.asarray(a, np.float32) for a in (Wq, Wk, Wv, Wo, bo))
    b, n, dim = x.shape
    assert (b, dim) == (1, DIM)

    pm = np.concatenate([np.array([True]), mask[0]])  # [n]
    sel = np.nonzero(pm)[0]
    C0 = len(sel)
    C = max(((C0 + 127) // 128) * 128, 1024)
    JB = C // 128
    assert C == 1024, f"schedule is specialized for C=1024, got C0={C0}"
    chunks = [(0, 512), (512, 384), (896, 128)]

    x_c = np.zeros((C, DIM), np.float32)
    x_c[:C0] = x[0][sel]
    m_c = np.zeros((C, DIM), np.float32)
    m_c[:C0] = m[0][sel]
    x_t = np.ascontiguousarray(x_c.T)  # [DIM, C]
    m_t = np.ascontiguousarray(m_c.T)

    x_sw = _swizzle_h(x_t, C)
    m_sw = _swizzle_h(m_t, C)

    jbias = np.zeros(C, np.float32)
    jbias[C0:] = -1e30
    jbias_t = np.ascontiguousarray(jbias.reshape(JB, 128).T)  # [128, JB]

    nc = _get_program(C, JB, chunks)

    in_maps = []
    for c in range(N_CORES):
        h0 = c * HPC * DH  # 128*c
        in_maps.append(
            {
                "x16": x_sw,
                "m16": m_sw,
                "wq": _swizzle_w(np.ascontiguousarray(Wq[:, h0 : h0 + 128])),
                "wk": _swizzle_w(np.ascontiguousarray(Wk[:, h0 : h0 + 128])),
                "wv": _swizzle_w(np.ascontiguousarray(Wv[:, h0 : h0 + 128])),
                "wo": np.ascontiguousarray(Wo[h0 : h0 + 128, :]).astype(np.float16),
                "jbias": jbias_t,
            }
        )

    res = run_bass_kernel_spmd(
        nc, in_maps, core_ids=list(range(N_CORES)), trace=_trace
    )
    if _bass_results is not None:
        _bass_results.append(res)

    acc = np.sum(
        np.stack([np.asarray(r["out"])[:C0].astype(np.float64) for r in res.results]),
        axis=0,
    )

    # host-side: masked rows get uniform attention over ALL positions
    mv = m[0].astype(np.float64).mean(axis=0)  # mean over all j of m
    mv_out = (mv @ Wv.astype(np.float64)) @ Wo.astype(np.float64)  # [dim]

    out = np.empty((n, DIM), np.float64)
    out[sel] = acc
    out[~pm] = mv_out
    out += bo.astype(np.float64)
    return out[None].astype(np.float32)
